# revision 5
# baseline (speedup 1.0000x reference)
"""Correlation cost-volume kernel for Trainium2 (8 NeuronCores).

out[b, dy*9+dx, y, x] = mean_c input1[b,c,y,x] * pad(input2)[b,c,y+dy,x+dx]

Sharding: pure data parallel over batch (B=8 -> 1 batch element per core).

Per core: 2D Gram tiling with 4-stacked small tiles. A "stack" covers an
8y x 16x output region split into four 8y x 4x sub-tiles (M=32 each). The
four sub-tile matmuls are col-tiled onto the PE array (tile_position=
(0,32j)) and run concurrently, each streaming its own 16y x 12x fp16 halo
(N=192) from in2p:
  G[p=(j,yy,xx), (y',x')] = sum_c in1[c,8g+yy,16t+4j+xx]
                                 * in2p[c, 8g+y', 16t+4j+x']
Small tiles keep the halo re-streaming low (192 cols per 32 outputs =
2.37x amplification) while the 4-stack fills all 128 PSUM partitions, so
the PSUM->SBUF scaled copy (x 1/C, alternating DVE/ACT, cast to bf16)
wastes no lanes. The dump is then a full contiguous SBUF row per
partition (no window selection; host extracts the 9x9 band in numpy
during unshard) -> 16 dumps of 1.5 KB-run descriptors, 12.6 MB total.

Inputs are cast to fp16 on the host (halves input HBM traffic; PSUM
accumulates fp32; rel err ~3e-4). in1 is pre-reordered stack-major so
each sub-tile's lhsT is a contiguous [C, 32] block.

The toolchain here rejects instructions with >1 sync wait, so after
tracing we split extra waits onto same-engine NoOps (split_multi_waits).
"""
import numpy as np

B, C, H, W = 8, 128, 128, 256
PAD = 4
ND = 9                   # displacements per axis
TY, TX = 8, 4            # sub-tile output (y, x)
NJ = 4                   # sub-tiles per stack
HY, HX = TY + 8, TX + 8  # sub-tile halo 16 x 12
NG = HY * HX             # 192 halo cols per sub-tile
NPH, NPW = H + 2 * PAD, W + 2 * PAD  # padded in2: 136 x 264
NGY, NGX = H // TY, W // (TX * NJ)   # 16 x 16 stacks
KT = NGX                 # stacks per dump group (one g row) = 16
SL = KT * NG             # sall row length: 3072

_CACHE = {}


def _build(split_waits=True):
    import concourse.bass as bass
    import concourse.mybir as mybir
    import bass_rust
    from concourse.ap import AP
    from concourse.tile import TileContext

    f32 = mybir.dt.float32
    f16 = mybir.dt.float16
    bf16 = mybir.dt.bfloat16

    nc = bass.Bass()
    IN1R = nc.dram_tensor("in1r", [C, H * W], f16, kind="ExternalInput")
    IN2P = nc.dram_tensor("in2p", [C, NPH * NPW], f16, kind="ExternalInput")
    OUTD = nc.dram_tensor("outd", [NGY, 128, SL], bf16, kind="ExternalOutput")

    with TileContext(nc) as tc:
        with tc.tile_pool(name="pin1", bufs=4) as pin1, \
             tc.tile_pool(name="pin2", bufs=4) as pin2, \
             tc.tile_pool(name="psum", bufs=4, space="PSUM") as psum, \
             tc.tile_pool(name="ps", bufs=2) as ps_s:
            # 4 y-chunks of inputs; chunk q serves stack rows g in [4q, 4q+4)
            t1c, t2c = [], []
            for q in range(4):
                t1 = pin1.tile([C, 4 * TY * W], f16)      # 8192 els
                nc.gpsimd.dma_start(
                    out=t1[:], in_=IN1R[:, q * 8192:(q + 1) * 8192])
                t1c.append(t1)
                t2 = pin2.tile([C, 40 * NPW], f16)        # rows [32q, 32q+40)
                nc.gpsimd.dma_start(
                    out=t2[:], in_=IN2P[:, 32 * q * NPW:(32 * q + 40) * NPW])
                t2c.append(t2)

            for g in range(NGY):
                q = g // 4
                t1t, t1o = t1c[q][:].tensor, t1c[q][:].offset
                t2t, t2o = t2c[q][:].tensor, t2c[q][:].offset
                r0 = (8 * g - 32 * q) * NPW  # halo row base within chunk
                sall = ps_s.tile([128, SL], bf16)
                st, so = sall[:].tensor, sall[:].offset
                for t in range(KT):
                    ps = psum.tile([128, NG], f32)
                    pt, po = ps[:].tensor, ps[:].offset
                    for j in range(NJ):
                        lhsT = AP(t1t,
                                  t1o + (g % 4) * 2048 + (t * NJ + j) * 32,
                                  [[4 * TY * W, C], [1, 32]])
                        rhs = AP(t2t, t2o + r0 + 16 * t + 4 * j,
                                 [[40 * NPW, C], [NPW, HY], [1, HX]])
                        out = AP(pt, po + 32 * j * NG, [[NG, 32], [1, NG]])
                        nc.tensor.matmul(out, lhsT, rhs, start=True,
                                         stop=True, tile_position=(0, 32 * j))
                    dst = AP(st, so + t * NG, [[SL, 128], [1, NG]])
                    if t % 2 == 0:
                        nc.scalar.mul(dst, ps[:], 1.0 / C)
                    else:
                        nc.vector.tensor_scalar_mul(dst, ps[:], 1.0 / C)
                dd = OUTD[g]
                ddst = AP(dd.tensor, dd.offset, [[SL, 128], [1, SL]])
                nc.sync.dma_start(out=ddst, in_=sall[:])

    # --- split multi-wait instructions (this walrus accepts max 1) ---
    if not split_waits:
        return nc
    n = 0
    for fn in nc.m.functions:
        for blk in fn.blocks:
            il = blk.instructions
            new = []
            changed = False
            for ins in il:
                si = ins.sync_info
                if si is not None and len(si.on_wait) > 1:
                    waits = list(si.on_wait)
                    for w in waits[:-1]:
                        n += 1
                        new.append(bass_rust.InstNoOp(
                            name=f"wsplit_{n}", engine=ins.engine,
                            sync_info=bass_rust.SyncInfo(
                                on_wait=[w], on_update=[])))
                    si.on_wait = waits[-1:]
                    ins.sync_info = si
                    changed = True
                new.append(ins)
            if changed:
                blk.instructions = new
    return nc


def _get_nc():
    if "nc" not in _CACHE:
        _CACHE["nc"] = _build()
    return _CACHE["nc"]


def prep_inputs(input1_b: np.ndarray, input2_b: np.ndarray) -> dict:
    """Host-side prep for one batch element: fp16 cast, in1 stack-major
    reorder [c, g, t, j, yy, xx], in2 padding."""
    x1 = input1_b.astype(np.float16)
    r = x1.reshape(C, NGY, TY, NGX, NJ, TX)        # [c, g, yy, t, j, xx]
    in1r = np.ascontiguousarray(
        r.transpose(0, 1, 3, 4, 2, 5)).reshape(C, H * W)
    in2p = np.pad(input2_b, ((0, 0), (PAD, PAD), (PAD, PAD))).astype(np.float16)
    return {"in1r": in1r,
            "in2p": np.ascontiguousarray(in2p.reshape(C, NPH * NPW))}


_YY = np.arange(TY)[:, None, None, None]
_XX = np.arange(TX)[None, :, None, None]
_DY = np.arange(ND)[None, None, :, None]
_DX = np.arange(ND)[None, None, None, :]


def decode_out(outd: np.ndarray) -> np.ndarray:
    """[16, 128, 3072] bf16 dump -> [81, H, W] fp32 output for one batch."""
    D = np.asarray(outd).astype(np.float32).reshape(
        NGY, NJ, TY, TX, KT, HY, HX)               # [g, j, yy, xx, t, y', x']
    D2 = D.transpose(0, 4, 1, 2, 3, 5, 6)          # [g, t, j, yy, xx, y', x']
    band = D2[:, :, :, _YY, _XX, _YY + _DY, _XX + _DX]  # [g,t,j,yy,xx,dy,dx]
    ob = band.transpose(5, 6, 0, 3, 1, 2, 4)       # [dy, dx, g, yy, t, j, xx]
    return np.ascontiguousarray(ob).reshape(ND * ND, H, W)


def kernel(input1: np.ndarray, input2: np.ndarray) -> np.ndarray:
    from concourse.bass_utils import run_bass_kernel_spmd

    input1 = np.ascontiguousarray(input1, dtype=np.float32)
    input2 = np.ascontiguousarray(input2, dtype=np.float32)
    in_maps = [prep_inputs(input1[b], input2[b]) for b in range(B)]

    nc = _get_nc()
    results = run_bass_kernel_spmd(nc, in_maps, core_ids=list(range(B))).results

    out = np.empty((B, ND * ND, H, W), dtype=np.float32)
    for b in range(B):
        out[b] = decode_out(results[b]["outd"])
    return out
